# revision 7
# baseline (speedup 1.0000x reference)
"""Trainium2 Bass kernel for MultiHeadSelfAttentionWithRPE.

Strategy (per core; data-parallel over batch N=8 across 8 cores):
  - rel = pe[idx] @ Wpos has only 1023 distinct rows -> compute
    R^T = Wpos^T @ pe^T (reversed) once per core: (768, 1023).
  - bd[q,k] = q2[q] . R[q-k+511] computed as windowed S matmuls
    (128, 639) per q-tile, then the relative-shift gather is a single
    SBUF->SBUF DMA with a diagonal access pattern (stride row_w-1).
  - All matmuls in float32r (1 cyc/row, ~1e-4 rel err).
  - softmax: exp on ACT with fused row-sum (accum_out); normalization
    folded into the PE attention transpose via diag(1/rowsum).
  - biases: per-partition ACT bias where the bias is along the partition
    dim; K=1 ones-matmul accumulation where it is along the free dim.
"""
import math
import os
import numpy as np

DBG = bool(int(os.environ.get("KBG_DEBUG", "0")))

L = 512
E = 768
H = 12
D = 64
MAXLEN = 512
R = 2 * MAXLEN - 1  # 1023
RP = 1024          # padded R for even-N f32r matmuls
ET = E // 128       # 6
LT = L // 128       # 4
NCORES = 8

_BUILT = {}


def _sinusoid_table():
    pos = np.arange(R, dtype=np.float32)[:, None]
    div = np.exp(np.arange(0, E, 2, dtype=np.float32) * (-math.log(10000.0) / E))
    ang = pos * div.astype(np.float32)
    return np.stack([np.sin(ang), np.cos(ang)], axis=-1).reshape(R, E).astype(np.float32)


def _build():
    if "nc" in _BUILT:
        return _BUILT["nc"]
    import concourse.bass as bass
    import concourse.bacc as bacc
    import concourse.mybir as mybir
    import concourse.tile as tile

    f32 = mybir.dt.float32
    f32r = mybir.dt.float32r
    AFT = mybir.ActivationFunctionType
    ALU = mybir.AluOpType

    nc = bacc.Bacc("TRN2", target_bir_lowering=False, debug=False)

    xq_d = nc.dram_tensor("xq", [L, E], f32r, kind="ExternalInput")
    xk_d = nc.dram_tensor("xk", [L, E], f32r, kind="ExternalInput")
    xv_d = nc.dram_tensor("xv", [L, E], f32r, kind="ExternalInput")
    W_d = {
        name: nc.dram_tensor(name, [E, E], f32r, kind="ExternalInput")
        for name in ["Wq", "Wk", "Wv", "Wo", "Wpos"]
    }
    peT_d = nc.dram_tensor("peT", [E, RP], f32r, kind="ExternalInput")
    b1s_d = nc.dram_tensor("b1s", [128, ET], f32, kind="ExternalInput")
    b2s_d = nc.dram_tensor("b2s", [128, ET], f32, kind="ExternalInput")
    bk_d = nc.dram_tensor("bkt", [128, ET], f32, kind="ExternalInput")
    bpos_d = nc.dram_tensor("bpost", [128, ET], f32, kind="ExternalInput")
    bv_d = nc.dram_tensor("bvr", [1, E], f32r, kind="ExternalInput")
    bo_d = nc.dram_tensor("bor", [1, E], f32r, kind="ExternalInput")
    ones_d = nc.dram_tensor("ones1", [1, 128], f32r, kind="ExternalInput")
    eye_d = nc.dram_tensor("eye", [128, 128], f32r, kind="ExternalInput")
    out_d = nc.dram_tensor("out", [L, E], f32, kind="ExternalOutput")

    taps = {}

    def tap(name, ap):
        if not DBG:
            return
        shp = [ap.partition_size(), ap.free_size()]
        td = nc.dram_tensor(f"tap_{name}", shp, f32, kind="ExternalOutput")
        src_ap = ap if ap.dtype == f32 else ap.bitcast(f32)
        nc.sync.dma_start(td.ap(), src_ap)
        taps[name] = td

    # toggle between ACT and DVE for PSUM->SBUF evacuations
    _tog = [0]

    def copy_alt(dst, src):
        if _tog[0] % 2 == 0:
            nc.scalar.copy(dst, src)
        else:
            nc.vector.tensor_copy(dst, src)
        _tog[0] += 1

    with tile.TileContext(nc) as tc:
        with (
            tc.tile_pool(name="const", bufs=1) as pc,
            tc.tile_pool(name="persist", bufs=1) as pp,
        ):
            eye_s = pc.tile([128, 128], f32r)
            nc.sync.dma_start(eye_s[:], eye_d.ap())
            ones_s = pc.tile([1, 128], f32r)
            nc.sync.dma_start(ones_s[:], ones_d.ap())
            bv_s = pc.tile([1, E], f32r)
            nc.sync.dma_start(bv_s[:], bv_d.ap())
            bo_s = pc.tile([1, E], f32r)
            nc.sync.dma_start(bo_s[:], bo_d.ap())
            b1s_s = pc.tile([128, ET], f32)
            nc.sync.dma_start(b1s_s[:], b1s_d.ap())
            b2s_s = pc.tile([128, ET], f32)
            nc.sync.dma_start(b2s_s[:], b2s_d.ap())
            bk_s = pc.tile([128, ET], f32)
            nc.sync.dma_start(bk_s[:], bk_d.ap())
            bpos_s = pc.tile([128, ET], f32)
            nc.sync.dma_start(bpos_s[:], bpos_d.ap())
            rsum_s = pc.tile([128, H * LT], f32)
            rinv_s = pc.tile([128, H * LT], f32)
            nc.gpsimd.memset(rsum_s[:], 0.0)

            RT_s = pp.tile([128, ET * RP], f32r)
            q1T_s = pp.tile([128, ET * L], f32r)
            q2T_s = pp.tile([128, ET * L], f32r)
            kT_s = pp.tile([128, ET * L], f32r)
            v_s = pp.tile([128, LT * E], f32r)

            # ---------------- stages A-C: R^T, projections ----------------
            with (
                tc.tile_pool(name="stg", bufs=1) as pstg,
                tc.tile_pool(name="psAB", bufs=1, space="PSUM") as ppAB,
            ):
                def load_W(name):
                    w = pstg.tile([128, ET * E], f32r, tag="W", bufs=2,
                                  name=f"{name}_s")
                    nc.sync.dma_start(
                        w[:], W_d[name].ap().rearrange("(ei p) f -> p ei f", p=128))
                    return w

                # R^T = Wpos^T @ peT_rev + bpos
                peT_s = pstg.tile([128, ET * RP], f32r, bufs=1)
                nc.sync.dma_start(
                    peT_s[:], peT_d.ap().rearrange("(j p) c -> p j c", p=128))
                Wpos_s = load_W("Wpos")
                for j in range(ET):
                    for off, w in [(0, 512), (512, 512)]:
                        psr = ppAB.tile([128, 512], f32, tag="psproj", bufs=3)
                        for ei in range(ET):
                            nc.tensor.matmul(
                                psr[:, 0:w],
                                Wpos_s[:, ei * E + j * 128: ei * E + (j + 1) * 128],
                                peT_s[:, ei * RP + off: ei * RP + off + w],
                                start=(ei == 0), stop=(ei == ET - 1))
                        nc.scalar.activation(
                            RT_s[:, j * RP + off: j * RP + off + w], psr[:, 0:w],
                            AFT.Identity, bias=bpos_s[:, j:j + 1])

                def load_x(xd, name):
                    xs = pstg.tile([128, LT * E], f32r, tag="x", bufs=2, name=name)
                    nc.sync.dma_start(
                        xs[:], xd.ap().rearrange("(i p) f -> p i f", p=128))
                    return xs

                def transpose_x(xs, name):
                    xT = pstg.tile([128, ET * L], f32r, tag="xT", bufs=2, name=name)
                    for e in range(ET):
                        pst_ = ppAB.tile([128, 512], f32r, tag="pstr", bufs=2)
                        for i in range(LT):
                            nc.tensor.transpose(
                                pst_[:, i * 128:(i + 1) * 128],
                                xs[:, i * E + e * 128: i * E + (e + 1) * 128],
                                eye_s[:])
                        copy_alt(xT[:, e * L: e * L + 512], pst_[:].bitcast(f32))
                    return xT

                # q projection (-> q1T, q2T with fused 1/sqrt(D) scale + biases)
                Wq_s = load_W("Wq")
                xqT = transpose_x(load_x(xq_d, "xq_s"), "xqT")
                tap("xqT", xqT[:])
                for eo in range(ET):
                    psq = ppAB.tile([128, 512], f32, tag="psproj", bufs=3)
                    for ei in range(ET):
                        nc.tensor.matmul(
                            psq[:],
                            Wq_s[:, ei * E + eo * 128: ei * E + (eo + 1) * 128],
                            xqT[:, ei * L: ei * L + 512],
                            start=(ei == 0), stop=(ei == ET - 1))
                    nc.scalar.activation(
                        q1T_s[:, eo * L: eo * L + 512], psq[:],
                        AFT.Identity, bias=b1s_s[:, eo:eo + 1], scale=0.125)
                    nc.vector.tensor_scalar(
                        q2T_s[:, eo * L: eo * L + 512], psq[:],
                        0.125, b2s_s[:, eo:eo + 1],
                        op0=ALU.mult, op1=ALU.add)

                # k projection
                Wk_s = load_W("Wk")
                xkT = transpose_x(load_x(xk_d, "xk_s"), "xkT")
                for eo in range(ET):
                    psk = ppAB.tile([128, 512], f32, tag="psproj", bufs=3)
                    for ei in range(ET):
                        nc.tensor.matmul(
                            psk[:],
                            Wk_s[:, ei * E + eo * 128: ei * E + (eo + 1) * 128],
                            xkT[:, ei * L: ei * L + 512],
                            start=(ei == 0), stop=(ei == ET - 1))
                    nc.scalar.activation(
                        kT_s[:, eo * L: eo * L + 512], psk[:],
                        AFT.Identity, bias=bk_s[:, eo:eo + 1])

                # v projection (natural layout, bias via K=1 ones matmul)
                Wv_s = load_W("Wv")
                xvT = transpose_x(load_x(xv_d, "xv_s"), "xvT")
                for i in range(LT):
                    for off, w in [(0, 512), (512, 256)]:
                        psv = ppAB.tile([128, 512], f32, tag="psproj", bufs=3)
                        for ei in range(ET):
                            nc.tensor.matmul(
                                psv[:, 0:w],
                                xvT[:, ei * L + i * 128: ei * L + (i + 1) * 128],
                                Wv_s[:, ei * E + off: ei * E + off + w],
                                start=(ei == 0), stop=False)
                        nc.tensor.matmul(
                            psv[:, 0:w], ones_s[:], bv_s[0:1, off:off + w],
                            start=False, stop=True)
                        copy_alt(v_s[:, i * E + off: i * E + off + w], psv[:, 0:w])

            tap("RT", RT_s[:])
            tap("q1T", q1T_s[:])
            tap("q2T", q2T_s[:])
            tap("kT", kT_s[:])
            tap("v", v_s[:])

            # ---------------- stage D: attention ----------------
            with (
                tc.tile_pool(name="attn", bufs=1) as pd,
                tc.tile_pool(name="psD", bufs=1, space="PSUM") as ppD,
            ):
                Wo_s = pd.tile([128, ET * E], f32r, bufs=1)
                nc.sync.dma_start(
                    Wo_s[:], W_d["Wo"].ap().rearrange("(ei p) f -> p ei f", p=128))
                avT_s = pd.tile([128, ET * L], f32r, bufs=1)

                for jb in range(H // 2):
                    for hh in range(2):
                        h = 2 * jb + hh
                        po = 64 * hh
                        exp_tiles = []
                        for i in range(LT):
                            c0 = 384 - 128 * i
                            # S matmuls: (128 q, 640 c) window
                            ps_s = ppD.tile([128, 640], f32, tag="pss", bufs=1)
                            q2blk = q2T_s[po:po + 64,
                                          jb * L + i * 128: jb * L + (i + 1) * 128]
                            nc.tensor.matmul(
                                ps_s[:, 0:512], q2blk,
                                RT_s[po:po + 64, jb * RP + c0: jb * RP + c0 + 512],
                                start=True, stop=True, tile_position=(po, 0))
                            nc.tensor.matmul(
                                ps_s[:, 512:640], q2blk,
                                RT_s[po:po + 64,
                                     jb * RP + c0 + 512: jb * RP + c0 + 640],
                                start=True, stop=True, tile_position=(po, 0))
                            s_sb = pd.tile([128, 640], f32, tag="ssb", bufs=3)
                            nc.scalar.copy(s_sb[:, 0:512], ps_s[:, 0:512])
                            nc.vector.tensor_copy(s_sb[:, 512:640],
                                                  ps_s[:, 512:640])
                            # relative-shift gather: diagonal SBUF->SBUF DMA
                            bd = pd.tile([128, 512], f32r, tag="bd", bufs=3)
                            nc.sync.dma_start(
                                bd[:],
                                bass.AP(s_sb.tensor, 127,
                                        [[639, 128], [1, 512]]).bitcast(f32r))
                            # scores = ac + bd (identity-matmul accumulate)
                            ps_sc = ppD.tile([128, 512], f32, tag="pssc", bufs=2)
                            nc.tensor.matmul(
                                ps_sc[:],
                                q1T_s[po:po + 64,
                                      jb * L + i * 128: jb * L + (i + 1) * 128],
                                kT_s[po:po + 64, jb * L: jb * L + 512],
                                start=True, stop=False, tile_position=(po, 0))
                            nc.tensor.matmul(
                                ps_sc[:], eye_s[:], bd[:],
                                start=False, stop=True)
                            expt = pd.tile([128, 512], f32r, tag="exp", bufs=8,
                                           name=f"exp_{h}_{i}")
                            col = h * LT + i
                            nc.scalar.activation(
                                expt[:], ps_sc[:], AFT.Exp,
                                accum_out=rsum_s[:, col:col + 1])
                            exp_tiles.append(expt)
                            if h <= 1 and i == 0:
                                tap(f"s_sb_{h}", s_sb[:])
                                tap(f"bd_{h}", bd[:])
                            if h == 0:
                                tap(f"exp_{i}", expt[:])

                        # 1/rowsum for this head; normalize exp -> attn
                        nc.vector.reciprocal(
                            rinv_s[:, h * LT:(h + 1) * LT],
                            rsum_s[:, h * LT:(h + 1) * LT])
                        attn_tiles = []
                        for i in range(LT):
                            at = pd.tile([128, 512], f32r, tag="attn", bufs=8,
                                         name=f"attn_{h}_{i}")
                            nc.vector.tensor_scalar_mul(
                                at[:], exp_tiles[i][:].bitcast(f32),
                                rinv_s[:, h * LT + i: h * LT + i + 1])
                            attn_tiles.append(at)
                        # attn^T via PE transpose (pure data movement)
                        attnT = pd.tile([128, LT * 512], f32r, tag="attnT",
                                        bufs=2, name=f"attnT_{h}")
                        for kt in range(LT):
                            ps_t = ppD.tile([128, 512], f32r, tag="pst", bufs=2)
                            for i in range(LT):
                                nc.tensor.transpose(
                                    ps_t[:, i * 128:(i + 1) * 128],
                                    attn_tiles[i][:, kt * 128:(kt + 1) * 128],
                                    eye_s[:])
                            copy_alt(attnT[:, kt * 512:(kt + 1) * 512],
                                     ps_t[:].bitcast(f32))
                        if h == 0:
                            tap("attnT0", attnT[:])
                        # av^T: (64 d, 512 q)
                        ps_av = ppD.tile([64, 512], f32, tag="psav", bufs=2)
                        for kt in range(LT):
                            nc.tensor.matmul(
                                ps_av[:],
                                v_s[:, kt * E + h * 64: kt * E + (h + 1) * 64],
                                attnT[:, kt * 512:(kt + 1) * 512],
                                start=(kt == 0), stop=(kt == LT - 1))
                        if hh == 0:
                            copy_alt(avT_s[0:64, jb * L: jb * L + 512], ps_av[:])
                        else:
                            avh = pd.tile([64, 512], f32r, tag="avh", bufs=2)
                            copy_alt(avh[:], ps_av[:])
                            nc.sync.dma_start(
                                avT_s[64:128, jb * L: jb * L + 512], avh[:])

                tap("rsum", rsum_s[:])
                tap("rinv", rinv_s[:])
                tap("avT", avT_s[:])

                # final projection: out = avT^T @ Wo + bo
                for i in range(LT):
                    osb = pd.tile([128, E], f32, tag="osb", bufs=2)
                    for off, w in [(0, 512), (512, 256)]:
                        psf = ppD.tile([128, 512], f32, tag="pssc", bufs=2)
                        for j in range(ET):
                            nc.tensor.matmul(
                                psf[:, 0:w],
                                avT_s[:, j * L + i * 128: j * L + (i + 1) * 128],
                                Wo_s[:, j * E + off: j * E + off + w],
                                start=(j == 0), stop=False)
                        nc.tensor.matmul(
                            psf[:, 0:w], ones_s[:], bo_s[0:1, off:off + w],
                            start=False, stop=True)
                        copy_alt(osb[:, off:off + w], psf[:, 0:w])
                    nc.sync.dma_start(out_d.ap()[i * 128:(i + 1) * 128, :], osb[:])

    nc.compile()
    _BUILT["nc"] = nc
    _BUILT["taps"] = taps
    return nc


def _host_inputs(inputs):
    """Shared (per-core-identical) device inputs from the full input dict."""
    f = lambda name: np.ascontiguousarray(np.asarray(inputs[name], dtype=np.float32))
    pe = _sinusoid_table()
    peT_rev = np.zeros((E, RP), np.float32)
    peT_rev[:, :R] = pe[::-1].T  # (E, R) reversed; col R zero-padded
    bq = f("bq")
    b1 = 0.125 * (bq + f("r_w_bias").reshape(E))
    b2 = 0.125 * (bq + f("r_r_bias").reshape(E))
    tcol = lambda v: np.ascontiguousarray(v.reshape(ET, 128).T.astype(np.float32))
    return {
        "Wq": f("Wq"), "Wk": f("Wk"), "Wv": f("Wv"), "Wo": f("Wo"),
        "Wpos": f("Wpos"),
        "peT": peT_rev,
        "b1s": tcol(b1), "b2s": tcol(b2),
        "bkt": tcol(f("bk")), "bpost": tcol(f("bpos")),
        "bvr": f("bv").reshape(1, E), "bor": f("bo").reshape(1, E),
        "ones1": np.ones((1, 128), np.float32),
        "eye": np.eye(128, dtype=np.float32),
    }


def _install_ntff_hook():
    import sys, types
    if "antenv.axon_hooks" in sys.modules:
        return
    mod = types.ModuleType("antenv.axon_hooks")
    _h = [None]
    mod.set_axon_ntff_profile_hook = lambda h: _h.__setitem__(0, h)
    mod.get_axon_ntff_profile_hook = lambda: _h[0]
    sys.modules["antenv.axon_hooks"] = mod
    import antenv
    antenv.axon_hooks = mod
    try:
        from trn_agent_boot.trn_boot import _ntff_profile_via_ctypes
        hook = _ntff_profile_via_ctypes("/opt/axon/libaxon_pjrt.so")
        if hook is not None:
            mod.set_axon_ntff_profile_hook(hook)
    except Exception as e:
        print("ntff hook install failed:", e)


def _run(inputs, trace=False):
    from concourse import bass_utils
    if trace:
        _install_ntff_hook()
    nc = _build()
    shared = _host_inputs(inputs)
    query = np.asarray(inputs["query"], dtype=np.float32)
    keys = np.asarray(inputs["keys"], dtype=np.float32)
    values = np.asarray(inputs["values"], dtype=np.float32)
    in_maps = [
        dict(shared,
             xq=np.ascontiguousarray(query[c]),
             xk=np.ascontiguousarray(keys[c]),
             xv=np.ascontiguousarray(values[c]))
        for c in range(NCORES)
    ]
    res = bass_utils.run_bass_kernel_spmd(
        nc, in_maps, core_ids=list(range(NCORES)), trace=trace)
    out = np.stack([res.results[c]["out"] for c in range(NCORES)])
    return out.astype(np.float32), res


def kernel(**inputs):
    out, _ = _run(inputs, trace=False)
    return out
